# revision 25
# baseline (speedup 1.0000x reference)
"""DenseAttention (causal quadratic variant, no softmax) — TRN2 Bass kernel.

Problem: out[b] = (tril(Q @ K^T) @ V) per head, where
  Q = X @ Wq (split into 16 heads of 64), K = V = X head slices.
Shapes: X [2, 2048, 1024] fp32, Wq [1024, 1024] fp32 -> out [2, 2048, 1024] fp32.

Sharding (8 cores): core c -> batch b = c//4, head group g = c%4 (4 heads,
output columns [256g, 256g+256)).  The queries projection is column-sharded
by head group; no cross-device communication.

Algorithm per core (linear-attention prefix-sum form, per head h), with
256-row outer query blocks t (T2 = 8), 128-row key blocks j:
  attnT_t = S_{<t}^T @ Q_t^T  +  sum_j V_j^T @ tril-part(K_j Q_t^T)
  S_t = S_{<t} + sum_{j in t} K_j^T V_j          ([64,64] state per head)
This reduces the strictly-causal off-diagonal work from O(N^2 hd) to
O(N hd^2).  The two heads of a "pair" p (one 128-partition group) share the
global matmul: their [64,64] states sit in a block-diagonal [128,128]
stationary tile (built by one DVE masked copy from the running Gram), so
the global term is ONE 128-contraction matmul per (t, p).  Everything is
computed transposed (attnT [hd, N]) so both matmul stages feed the tensor
engine without on-device transposes; the host un-transposes.  All matmuls
run in bf16 with fp32 PSUM accumulation; the output ships bf16.

PSUM rules honoured (hardware-verified): matmul groups writing the same
PSUM bank must share one accumulation group whose first (start=True) write
covers the region; two start=True groups split by column ranges in one
bank fault the exec unit.  Row(partition)-splits are fine.
"""

import numpy as np
import ml_dtypes

import concourse.bacc as bacc
import concourse.mybir as mybir
import concourse.tile as tile
from concourse import bass_utils
from concourse.bass import ds

B, N, D = 2, 2048, 1024
H, HD = 16, 64
NCORES = 8
P = 128           # partition dim == key block size
NQ = 256          # outer query block size
T2 = N // NQ      # 8 outer blocks
KB = N // P       # 16 key blocks
CW = 256          # per-core output column width (4 heads = 2 pairs)
CHUNK = 512       # xt chunk width

DT = mybir.dt.bfloat16
NPDT = ml_dtypes.bfloat16
F32 = mybir.dt.float32


def _emit(nc, tc, pools, xt_d, wq_d, xv_d, mk_d, out_d, deep=1):
    cpool, wpool, psq, psst, psat, pss = pools

    # ---------------- input DMAs: few, large, fully-contiguous transfers,
    # ALL on the SP (sync) HWDGE queue.  SP runs no compute, so in a looped
    # build the next emission's input DMAs issue right behind this one's
    # and prefetch during compute.  (Outputs ride the ACT queue: they are
    # produced late anyway, and must not delay the next emission's inputs.)
    wqall = cpool.tile([P, 8 * CW], DT, name="wqall", tag="wqall", bufs=deep)
    nc.sync.dma_start(out=wqall, in_=wq_d)

    xvall = cpool.tile([P, KB * CW], DT, name="xvall", tag="xvall", bufs=deep)
    xtall = cpool.tile([P, 8 * N], DT, name="xtall", tag="xtall", bufs=deep)
    mk_sb = cpool.tile([P, 512], DT, name="mk_sb", tag="mk_sb", bufs=deep)

    nc.sync.dma_start(out=xtall[:, ds(0, 4096)], in_=xt_d[:, ds(0, 4096)])
    nc.sync.dma_start(out=mk_sb, in_=mk_d)
    mk_o1 = mk_sb[:, ds(0, P)]      # tril           (score masks)
    mk_bd = mk_sb[:, ds(NQ, 2 * P)]  # block-diag x2  (state snapshot mask)

    for h in range(2):
        nc.sync.dma_start(
            out=xvall[:, ds(1024 * h, 1024)],
            in_=xv_d[:, ds(1024 * h, 1024)],
        )
    nc.sync.dma_start(out=xtall[:, ds(4096, 4096)], in_=xt_d[:, ds(4096, 4096)])
    for h in range(2, 4):
        nc.sync.dma_start(
            out=xvall[:, ds(1024 * h, 1024)],
            in_=xv_d[:, ds(1024 * h, 1024)],
        )
    # xt chunks 2-3 ride the ACT queue: their configs sit behind the
    # previous emission's Act compute, so they issue around this emission's
    # start -- early enough for data needed at its back half, and they
    # halve the SP queue's latency-critical stream.
    for c in range(2, 4):
        nc.scalar.dma_start(
            out=xtall[:, ds(4096 * c, 4096)],
            in_=xt_d[:, ds(4096 * c, 4096)],
        )

    def kt_ap(j, p, e):
        # K^T for (key block j, pair p, head e): [64 dims, 128 keys]
        c_, wo = divmod(P * j, CHUNK)
        return xtall[ds(HD * e, HD), ds(4096 * c_ + CHUNK * p + wo, P)]

    def xv_ap(j, col, w):
        return xvall[:, ds(CW * j + col, w)]

    qt_sb = [cpool.tile([P, N], DT, name=f"qt{m}", tag=f"qt{m}") for m in range(2)]
    otall = [cpool.tile([P, N], DT, name=f"ot{p}", tag=f"ot{p}") for p in range(2)]

    # ---------------- S states: running prefix Grams, one [128,128] matmul
    # per (pair, key block j): X_j^T @ X_j accumulated in PSUM (one tile per
    # pair -- the two pairs' groups must not share a PSUM bank).  After each
    # odd j one DVE masked-copy per pair snapshots the block-diagonal
    # [64,64] head states into a [128,128] bf16 tile (cross-head blocks
    # zeroed by the mask) for the single-matmul global term.  Grams are
    # interleaved into the main loop with a lead of ~2 key blocks so the
    # serial Gram -> snap -> Gram WAR chain always has PE work as slack.
    snaps = []   # snaps[s][p]: state over keys < 256(s+1)
    spsb = pss.tile([P, 2 * P], F32, name="spsb", tag="spsb")
    nc.vector.memset(spsb, 0.0)
    sps = [spsb[:, ds(P * p, P)] for p in range(2)]

    def emit_gram(j):
        if j > KB - 3:
            return   # blocks 14,15 are never snapshotted (diag covers them)
        for p in range(2):
            v = xv_ap(j, P * p, P)
            nc.tensor.matmul(
                sps[p], v, v, start=False, stop=(j == KB - 3),
                skip_group_check=True,
            )
        if j % 2 == 1 and j < KB - 1:
            s = j // 2
            snap = cpool.tile([P, 2 * P], DT, name=f"sn{s}", tag=f"sn{s}")
            nc.vector.tensor_mul(snap, spsb, mk_bd)
            snaps.append(snap)

    def emit_st(t):
        # scores for outer block t: per (pair, head) an [128,256] o=0 tile
        # (keys 2t, queries masked tril|dense) and an [128,128] o=1 tile
        # (keys 2t+1, tril).  Masks fuse into the PSUM->SBUF copies.
        out = []
        for p in range(2):
            for e in range(2):
                stsb = wpool.tile([P, NQ + P], DT, name=f"st{t}_{p}_{e}",
                                  tag="st", bufs=6)
                st0 = psst.tile([P, NQ], F32, name=f"s0_{t}_{p}_{e}",
                                tag="st0", bufs=2)
                qv0 = qt_sb[p][ds(HD * e, HD), ds(NQ * t, NQ)]
                nc.tensor.matmul(st0, kt_ap(2 * t, p, e), qv0,
                                 start=True, stop=True, skip_group_check=True)
                st1 = psst.tile([P, P], F32, name=f"s1_{t}_{p}_{e}",
                                tag="st1", bufs=2)
                qv1 = qt_sb[p][ds(HD * e, HD), ds(NQ * t + P, P)]
                nc.tensor.matmul(st1, kt_ap(2 * t + 1, p, e), qv1,
                                 start=True, stop=True, skip_group_check=True)
                nc.vector.tensor_mul(stsb[:, ds(0, P)], st0[:, ds(0, P)], mk_o1)
                nc.scalar.copy(stsb[:, ds(P, P)], st0[:, ds(P, P)])
                nc.vector.tensor_mul(stsb[:, ds(NQ, P)], st1, mk_o1)
                out.append((p, e, stsb))
        return out

    def emit_pv(t, sts):
        ats = {}
        for p in range(2):
            at = psat.tile([P, NQ], F32, name=f"at{t}_{p}", tag="at", bufs=2)
            ats[p] = at
            if t > 0:
                # global term: attnT_t += S_{<t}^T @ Q_t^T (block-diag state,
                # both heads in one 128-contraction matmul; S symmetric)
                nc.tensor.matmul(
                    at, snaps[t - 1][:, ds(P * p, P)],
                    qt_sb[p][:, ds(NQ * t, NQ)],
                    start=True, stop=False, skip_group_check=True,
                )
        for p, e, stsb in sts:
            at = ats[p]
            # diagonal terms: attnT_t += V_j^T @ masked scores
            nc.tensor.matmul(
                at[ds(HD * e, HD), :],
                xv_ap(2 * t, P * p + HD * e, HD),
                stsb[:, ds(0, NQ)],
                start=(t == 0), stop=False,
                tile_position=(0, HD * e),
                skip_group_check=True,
            )
            nc.tensor.matmul(
                at[ds(HD * e, HD), ds(P, P)],
                xv_ap(2 * t + 1, P * p + HD * e, HD),
                stsb[:, ds(NQ, P)],
                start=False, stop=True,
                tile_position=(0, HD * e),
                skip_group_check=True,
            )
        for p in range(2):
            nc.scalar.copy(otall[p][:, ds(NQ * t, NQ)], ats[p])

    # ---------------- fused main loop over 512-column chunks c:
    #   Q-proj chunk c (both m halves), then outer blocks t=2c, 2c+1.
    # Two-stage pipeline: while the DVE masks block t's scores, the PE runs
    # block t-1's global/PV matmuls.  Grams run 4 key blocks ahead.
    pending = None
    for c in range(4):
        for m in range(2):
            qp = psq.tile([P, CHUNK], F32, name=f"qp{m}_{c}", tag="qp", bufs=1)
            for k in range(8):
                nc.tensor.matmul(
                    qp,
                    wqall[:, ds(CW * k + P * m, P)],
                    xtall[:, ds(4096 * c + CHUNK * k, CHUNK)],
                    start=(k == 0), stop=(k == 7),
                )
            nc.scalar.copy(qt_sb[m][:, ds(CHUNK * c, CHUNK)], qp)
        if c == 0:
            # prologue Grams (after Q-proj c0 so a looped next-iteration's
            # PE stream has slack against the previous iteration's last
            # snapshot read of the sps tiles)
            for j in range(4):
                emit_gram(j)

        for t in (2 * c, 2 * c + 1):
            sts = emit_st(t)
            for j in (2 * t + 4, 2 * t + 5):
                if j < KB:
                    emit_gram(j)
            if pending is not None:
                emit_pv(*pending)
            pending = (t, sts)

        if c > 0:
            for p in range(2):
                nc.gpsimd.dma_start(
                    out=out_d[ds(P * p, P), ds(CHUNK * (c - 1), CHUNK)],
                    in_=otall[p][:, ds(CHUNK * (c - 1), CHUNK)],
                )

    emit_pv(*pending)
    for p in range(2):
        nc.gpsimd.dma_start(
            out=out_d[ds(P * p, P), ds(CHUNK * 3, CHUNK)],
            in_=otall[p][:, ds(CHUNK * 3, CHUNK)],
        )


def build_nc(loop_n=1):
    nc = bacc.Bacc("TRN2", target_bir_lowering=False, debug=False)
    # all inputs ship pre-arranged in their SBUF layouts (see make_in_maps)
    xt_d = nc.dram_tensor("xt", [P, 8 * N], DT, kind="ExternalInput").ap()
    wq_d = nc.dram_tensor("wq", [P, 8 * CW], DT, kind="ExternalInput").ap()
    xv_d = nc.dram_tensor("xv", [P, KB * CW], DT, kind="ExternalInput").ap()
    mk_d = nc.dram_tensor("mk", [P, 512], DT, kind="ExternalInput").ap()
    out_d = nc.dram_tensor("outT", [CW, N], DT, kind="ExternalOutput").ap()
    unroll = 2 if loop_n > 1 else 1
    per_iter = 8 if (loop_n > 1 and loop_n % 8 == 0) else 2
    with tile.TileContext(nc) as tc:
        with (
            tc.tile_pool(name="const", bufs=unroll) as cpool,
            tc.tile_pool(name="work", bufs=1) as wpool,
            tc.tile_pool(name="psq", bufs=1, space="PSUM") as psq,
            tc.tile_pool(name="psst", bufs=1, space="PSUM") as psst,
            tc.tile_pool(name="psat", bufs=1, space="PSUM") as psat,
            tc.tile_pool(name="pss", bufs=1, space="PSUM") as pss,
        ):
            pools = (cpool, wpool, psq, psst, psat, pss)
            if loop_n > 1:
                # timing-only build: repeat the kernel on-device so the
                # per-iteration time excludes host/RPC overhead.  Two
                # emissions per For_i iteration: tag rotation gives each
                # its own buffers, so iteration i+1's input DMAs overlap
                # iteration i's compute.
                assert loop_n % per_iter == 0
                hints = (mybir.EngineType.PE, mybir.EngineType.DVE,
                         mybir.EngineType.Activation, mybir.EngineType.SP)
                with tc.For_i(0, loop_n // per_iter, 1, hint_engines=hints):
                    for _ in range(per_iter):
                        _emit(nc, tc, pools, xt_d, wq_d, xv_d, mk_d, out_d,
                              deep=2)
            else:
                _emit(nc, tc, pools, xt_d, wq_d, xv_d, mk_d, out_d)
    nc.compile()
    return nc


_CACHE = {}


def get_nc():
    if "nc" not in _CACHE:
        _CACHE["nc"] = build_nc()
    return _CACHE["nc"]


def make_in_maps(hidden_states, queries_weight):
    X = np.asarray(hidden_states, dtype=np.float32)
    W = np.asarray(queries_weight, dtype=np.float32)
    r = np.arange(P)[:, None]
    c = np.arange(NQ)[None, :]
    m0 = (c >= r).astype(np.float32)                        # [tril | ones]
    bd = np.zeros((P, P), dtype=np.float32)                 # block-diag ones
    bd[:HD, :HD] = 1.0
    bd[HD:, HD:] = 1.0
    mk = np.concatenate([m0, bd, bd], axis=1).astype(NPDT)  # [128, 512]
    in_maps = []
    for core in range(NCORES):
        b, g = divmod(core, 4)
        cols = slice(CW * g, CW * g + CW)
        # Permute the contraction rows so every core sees its own heads'
        # K^T rows at xt rows [0, 256) (keeps the program core-agnostic).
        perm = np.r_[
            np.arange(CW * g, CW * g + CW),
            np.arange(0, CW * g),
            np.arange(CW * g + CW, D),
        ]
        # pre-arrange into SBUF layouts so every DMA is fully contiguous:
        #   xt: [p, (chunk c, k-tile, w)], wq: [p, (k, w)], xv: [p, (j, w)]
        xt = (X[b].T[perm].reshape(8, P, 4, CHUNK).transpose(1, 2, 0, 3)
              .reshape(P, 8 * N))
        wq = W[perm][:, cols].reshape(8, P, CW).transpose(1, 0, 2).reshape(P, 8 * CW)
        xv = X[b][:, cols].reshape(KB, P, CW).transpose(1, 0, 2).reshape(P, KB * CW)
        in_maps.append({
            "xt": np.ascontiguousarray(xt).astype(NPDT),
            "wq": np.ascontiguousarray(wq).astype(NPDT),
            "xv": np.ascontiguousarray(xv).astype(NPDT),
            "mk": mk,
        })
    return in_maps


def assemble(results):
    out = np.empty((B, N, D), dtype=np.float32)
    for core in range(NCORES):
        b, g = divmod(core, 4)
        out[b, :, CW * g:CW * g + CW] = results[core]["outT"].astype(np.float32).T
    return out


def kernel(hidden_states, queries_weight):
    nc = get_nc()
    in_maps = make_in_maps(hidden_states, queries_weight)
    res = bass_utils.run_bass_kernel_spmd(nc, in_maps, core_ids=list(range(NCORES)))
    return assemble(res.results)


# revision 29
# speedup vs baseline: 1.3111x; 1.3111x over previous
"""DenseAttention (causal quadratic variant, no softmax) — TRN2 Bass kernel.

Problem: out[b] = (tril(Q @ K^T) @ V) per head, where
  Q = X @ Wq (split into 16 heads of 64), K = V = X head slices.
Shapes: X [2, 2048, 1024] fp32, Wq [1024, 1024] fp32 -> out [2, 2048, 1024] fp32.

Sharding (8 cores): core c -> batch b = c//4, head group g = c%4 (4 heads,
output columns [256g, 256g+256)).  The queries projection is column-sharded
by head group; no cross-device communication.

Algorithm per core (linear-attention prefix-sum form, per head h), with
256-row outer query blocks t (T2 = 8), 128-row key blocks j:
  attnT_t = S_{<t}^T @ Q_t^T  +  sum_j V_j^T @ tril-part(K_j Q_t^T)
  S_t = S_{<t} + sum_{j in t} K_j^T V_j          ([64,64] state per head)
This reduces the strictly-causal off-diagonal work from O(N^2 hd) to
O(N hd^2).  The two heads of a "pair" p (one 128-partition group) share the
global matmul: their [64,64] states sit in a block-diagonal [128,128]
stationary tile (built by one DVE masked copy from the running Gram), so
the global term is ONE 128-contraction matmul per (t, p).  Everything is
computed transposed (attnT [hd, N]) so both matmul stages feed the tensor
engine without on-device transposes; the host un-transposes.  All matmuls
run in bf16 with fp32 PSUM accumulation; the output ships bf16.

PSUM rules honoured (hardware-verified): matmul groups writing the same
PSUM bank must share one accumulation group whose first (start=True) write
covers the region; two start=True groups split by column ranges in one
bank fault the exec unit.  Row(partition)-splits are fine.
"""

import numpy as np
import ml_dtypes

import concourse.bacc as bacc
import concourse.mybir as mybir
import concourse.tile as tile
from concourse import bass_utils
from concourse.bass import ds

B, N, D = 2, 2048, 1024
H, HD = 16, 64
NCORES = 8
P = 128           # partition dim == key block size
NQ = 256          # outer query block size
T2 = N // NQ      # 8 outer blocks
KB = N // P       # 16 key blocks
CW = 256          # per-core output column width (4 heads = 2 pairs)
CHUNK = 512       # xt chunk width

DT = mybir.dt.bfloat16
NPDT = ml_dtypes.bfloat16
F32 = mybir.dt.float32


def _emit(nc, tc, pools, xt_d, wq_d, xv_d, mk_d, out_d, deep=1):
    cpool, wpool, psq, psst, psat, pss = pools

    # ---------------- input DMAs: few, large, fully-contiguous transfers,
    # ALL on the SP (sync) HWDGE queue.  SP runs no compute, so in a looped
    # build the next emission's input DMAs issue right behind this one's
    # and prefetch during compute.  (Outputs ride the ACT queue: they are
    # produced late anyway, and must not delay the next emission's inputs.)
    wqall = cpool.tile([P, 8 * CW], DT, name="wqall", tag="wqall", bufs=deep)
    nc.sync.dma_start(out=wqall, in_=wq_d)

    xvall = cpool.tile([P, KB * CW], DT, name="xvall", tag="xvall", bufs=deep)
    xtall = cpool.tile([P, 8 * N], DT, name="xtall", tag="xtall", bufs=deep)
    mk_sb = cpool.tile([P, 640], DT, name="mk_sb", tag="mk_sb", bufs=deep)

    nc.sync.dma_start(out=xtall[:, ds(0, 4096)], in_=xt_d[:, ds(0, 4096)])
    nc.sync.dma_start(out=mk_sb, in_=mk_d)
    mk_st = mk_sb[:, ds(0, NQ + P)]      # [tril|ones|tril] (score mask)
    mk_bd = mk_sb[:, ds(NQ + P, 2 * P)]  # block-diag x2 (snapshot mask)

    for h in range(2):
        nc.sync.dma_start(
            out=xvall[:, ds(1024 * h, 1024)],
            in_=xv_d[:, ds(1024 * h, 1024)],
        )
    nc.sync.dma_start(out=xtall[:, ds(4096, 4096)], in_=xt_d[:, ds(4096, 4096)])
    for h in range(2, 4):
        nc.sync.dma_start(
            out=xvall[:, ds(1024 * h, 1024)],
            in_=xv_d[:, ds(1024 * h, 1024)],
        )
    # xt chunks 2-3 ride the ACT queue: their configs sit behind the
    # previous emission's Act compute, so they issue around this emission's
    # start -- early enough for data needed at its back half, and they
    # halve the SP queue's latency-critical stream.
    for c in range(2, 4):
        nc.scalar.dma_start(
            out=xtall[:, ds(4096 * c, 4096)],
            in_=xt_d[:, ds(4096 * c, 4096)],
        )

    def kt_ap(j, p, e):
        # K^T for (key block j, pair p, head e): [64 dims, 128 keys]
        c_, wo = divmod(P * j, CHUNK)
        return xtall[ds(HD * e, HD), ds(4096 * c_ + CHUNK * p + wo, P)]

    def xv_ap(j, col, w):
        return xvall[:, ds(CW * j + col, w)]

    qt_sb = [cpool.tile([P, N], DT, name=f"qt{m}", tag=f"qt{m}") for m in range(2)]
    otall = [cpool.tile([P, N], DT, name=f"ot{p}", tag=f"ot{p}") for p in range(2)]

    # ---------------- S states: running prefix Grams, one [128,128] matmul
    # per (pair, key block j): X_j^T @ X_j accumulated in PSUM (one tile per
    # pair -- the two pairs' groups must not share a PSUM bank).  After each
    # odd j one DVE masked-copy per pair snapshots the block-diagonal
    # [64,64] head states into a [128,128] bf16 tile (cross-head blocks
    # zeroed by the mask) for the single-matmul global term.  Grams are
    # interleaved into the main loop with a lead of ~2 key blocks so the
    # serial Gram -> snap -> Gram WAR chain always has PE work as slack.
    snaps = []   # snaps[s][p]: state over keys < 256(s+1)
    spsb = pss.tile([P, 2 * P], F32, name="spsb", tag="spsb")
    nc.vector.memset(spsb, 0.0)
    sps = [spsb[:, ds(P * p, P)] for p in range(2)]

    def emit_gram(j):
        if j > KB - 3:
            return   # blocks 14,15 are never snapshotted (diag covers them)
        for p in range(2):
            v = xv_ap(j, P * p, P)
            nc.tensor.matmul(
                sps[p], v, v, start=False, stop=(j == KB - 3),
                skip_group_check=True,
            )
        if j % 2 == 1 and j < KB - 1:
            s = j // 2
            snap = cpool.tile([P, 2 * P], DT, name=f"sn{s}", tag=f"sn{s}")
            nc.vector.tensor_mul(snap, spsb, mk_bd)
            snaps.append(snap)

    def emit_st(t):
        # scores for outer block t, per (pair, head) ONE [128,384] PSUM tile:
        # cols 0:256 = keys 2t x queries (o=0, start=True), cols 256:384 =
        # keys 2t+1 x last-half queries (o=1).  o=1 rides the SAME
        # accumulation group with start=False: o=0's bank-wide has_written
        # clear leaves o=1's cols unwritten, so its start=False write is a
        # fresh write (HW-verified semantics), avoiding a second col-split
        # start in the bank (which faults).  One DVE masked copy
        # ([tril|ones|tril]) moves the whole tile to SBUF.
        out = []
        for p in range(2):
            for e in range(2):
                stsb = wpool.tile([P, NQ + P], DT, name=f"st{t}_{p}_{e}",
                                  tag="st", bufs=6)
                stp = psst.tile([P, NQ + P], F32, name=f"sp{t}_{p}_{e}",
                                tag="big", bufs=5)
                qv0 = qt_sb[p][ds(HD * e, HD), ds(NQ * t, NQ)]
                nc.tensor.matmul(stp[:, ds(0, NQ)], kt_ap(2 * t, p, e), qv0,
                                 start=True, stop=False, skip_group_check=True)
                qv1 = qt_sb[p][ds(HD * e, HD), ds(NQ * t + P, P)]
                nc.tensor.matmul(stp[:, ds(NQ, P)], kt_ap(2 * t + 1, p, e), qv1,
                                 start=False, stop=True, skip_group_check=True)
                nc.vector.tensor_mul(stsb, stp, mk_st)
                out.append((p, e, stsb))
        return out

    def emit_pv(t, sts):
        ats = {}
        for p in range(2):
            at = psat.tile([P, NQ], F32, name=f"at{t}_{p}", tag="at", bufs=2)
            ats[p] = at
            if t > 0:
                # global term: attnT_t += S_{<t}^T @ Q_t^T (block-diag state,
                # both heads in one 128-contraction matmul; S symmetric)
                nc.tensor.matmul(
                    at, snaps[t - 1][:, ds(P * p, P)],
                    qt_sb[p][:, ds(NQ * t, NQ)],
                    start=True, stop=False, skip_group_check=True,
                )
        for p, e, stsb in sts:
            at = ats[p]
            # diagonal terms: attnT_t += V_j^T @ masked scores
            nc.tensor.matmul(
                at[ds(HD * e, HD), :],
                xv_ap(2 * t, P * p + HD * e, HD),
                stsb[:, ds(0, NQ)],
                start=(t == 0), stop=False,
                tile_position=(0, HD * e),
                skip_group_check=True,
            )
            nc.tensor.matmul(
                at[ds(HD * e, HD), ds(P, P)],
                xv_ap(2 * t + 1, P * p + HD * e, HD),
                stsb[:, ds(NQ, P)],
                start=False, stop=True,
                tile_position=(0, HD * e),
                skip_group_check=True,
            )
        for p in range(2):
            nc.scalar.copy(otall[p][:, ds(NQ * t, NQ)], ats[p])

    # ---------------- fused main loop over 512-column chunks c.
    # Q-proj runs ONE CHUNK AHEAD of the ST/PV blocks so its Act PSUM->SBUF
    # qt copy has a full stage to land before the STs read it; qp shares
    # the 5-deep "big" PSUM tag with the score tiles, so neither ever
    # waits on a fresh rotation slot.  Two-stage ST->PV pipeline as before;
    # Grams run ~4 key blocks ahead of their global-term consumers.
    def emit_qproj(c):
        for m in range(2):
            qp = psst.tile([P, CHUNK], F32, name=f"qp{m}_{c}", tag="big",
                           bufs=5)
            for k in range(8):
                nc.tensor.matmul(
                    qp,
                    wqall[:, ds(CW * k + P * m, P)],
                    xtall[:, ds(4096 * c + CHUNK * k, CHUNK)],
                    start=(k == 0), stop=(k == 7),
                )
            nc.scalar.copy(qt_sb[m][:, ds(CHUNK * c, CHUNK)], qp)

    pending = None
    emit_qproj(0)
    for j in range(4):
        emit_gram(j)
    for c in range(4):
        if c + 1 < 4:
            emit_qproj(c + 1)
        for t in (2 * c, 2 * c + 1):
            sts = emit_st(t)
            for j in (2 * t + 4, 2 * t + 5):
                if j < KB:
                    emit_gram(j)
            if pending is not None:
                emit_pv(*pending)
            pending = (t, sts)

        if c > 0:
            for p in range(2):
                nc.gpsimd.dma_start(
                    out=out_d[ds(P * p, P), ds(CHUNK * (c - 1), CHUNK)],
                    in_=otall[p][:, ds(CHUNK * (c - 1), CHUNK)],
                )

    emit_pv(*pending)
    for p in range(2):
        nc.gpsimd.dma_start(
            out=out_d[ds(P * p, P), ds(CHUNK * 3, CHUNK)],
            in_=otall[p][:, ds(CHUNK * 3, CHUNK)],
        )


def build_nc(loop_n=1):
    nc = bacc.Bacc("TRN2", target_bir_lowering=False, debug=False)
    # all inputs ship pre-arranged in their SBUF layouts (see make_in_maps)
    xt_d = nc.dram_tensor("xt", [P, 8 * N], DT, kind="ExternalInput").ap()
    wq_d = nc.dram_tensor("wq", [P, 8 * CW], DT, kind="ExternalInput").ap()
    xv_d = nc.dram_tensor("xv", [P, KB * CW], DT, kind="ExternalInput").ap()
    mk_d = nc.dram_tensor("mk", [P, 640], DT, kind="ExternalInput").ap()
    out_d = nc.dram_tensor("outT", [CW, N], DT, kind="ExternalOutput").ap()
    unroll = 2 if loop_n > 1 else 1
    per_iter = 8 if (loop_n > 1 and loop_n % 8 == 0) else 2
    with tile.TileContext(nc) as tc:
        with (
            tc.tile_pool(name="const", bufs=unroll) as cpool,
            tc.tile_pool(name="work", bufs=1) as wpool,
            tc.tile_pool(name="psq", bufs=1, space="PSUM") as psq,
            tc.tile_pool(name="psst", bufs=1, space="PSUM") as psst,
            tc.tile_pool(name="psat", bufs=1, space="PSUM") as psat,
            tc.tile_pool(name="pss", bufs=1, space="PSUM") as pss,
        ):
            pools = (cpool, wpool, psq, psst, psat, pss)
            if loop_n > 1:
                # timing-only build: repeat the kernel on-device so the
                # per-iteration time excludes host/RPC overhead.  Two
                # emissions per For_i iteration: tag rotation gives each
                # its own buffers, so iteration i+1's input DMAs overlap
                # iteration i's compute.
                assert loop_n % per_iter == 0
                hints = (mybir.EngineType.PE, mybir.EngineType.DVE,
                         mybir.EngineType.Activation, mybir.EngineType.SP)
                with tc.For_i(0, loop_n // per_iter, 1, hint_engines=hints):
                    for _ in range(per_iter):
                        _emit(nc, tc, pools, xt_d, wq_d, xv_d, mk_d, out_d,
                              deep=2)
            else:
                _emit(nc, tc, pools, xt_d, wq_d, xv_d, mk_d, out_d)
    nc.compile()
    return nc


_CACHE = {}


def get_nc():
    if "nc" not in _CACHE:
        _CACHE["nc"] = build_nc()
    return _CACHE["nc"]


def make_in_maps(hidden_states, queries_weight):
    X = np.asarray(hidden_states, dtype=np.float32)
    W = np.asarray(queries_weight, dtype=np.float32)
    r = np.arange(P)[:, None]
    c = np.arange(NQ)[None, :]
    m0 = (c >= r).astype(np.float32)                        # [tril | ones]
    bd = np.zeros((P, P), dtype=np.float32)                 # block-diag ones
    bd[:HD, :HD] = 1.0
    bd[HD:, HD:] = 1.0
    tril = m0[:, :P]
    mk = np.concatenate([m0, tril, bd, bd], axis=1).astype(NPDT)  # [128, 640]
    in_maps = []
    for core in range(NCORES):
        b, g = divmod(core, 4)
        cols = slice(CW * g, CW * g + CW)
        # Permute the contraction rows so every core sees its own heads'
        # K^T rows at xt rows [0, 256) (keeps the program core-agnostic).
        perm = np.r_[
            np.arange(CW * g, CW * g + CW),
            np.arange(0, CW * g),
            np.arange(CW * g + CW, D),
        ]
        # pre-arrange into SBUF layouts so every DMA is fully contiguous:
        #   xt: [p, (chunk c, k-tile, w)], wq: [p, (k, w)], xv: [p, (j, w)]
        xt = (X[b].T[perm].reshape(8, P, 4, CHUNK).transpose(1, 2, 0, 3)
              .reshape(P, 8 * N))
        wq = W[perm][:, cols].reshape(8, P, CW).transpose(1, 0, 2).reshape(P, 8 * CW)
        xv = X[b][:, cols].reshape(KB, P, CW).transpose(1, 0, 2).reshape(P, KB * CW)
        in_maps.append({
            "xt": np.ascontiguousarray(xt).astype(NPDT),
            "wq": np.ascontiguousarray(wq).astype(NPDT),
            "xv": np.ascontiguousarray(xv).astype(NPDT),
            "mk": mk,
        })
    return in_maps


def assemble(results):
    out = np.empty((B, N, D), dtype=np.float32)
    for core in range(NCORES):
        b, g = divmod(core, 4)
        out[b, :, CW * g:CW * g + CW] = results[core]["outT"].astype(np.float32).T
    return out


def kernel(hidden_states, queries_weight):
    nc = get_nc()
    in_maps = make_in_maps(hidden_states, queries_weight)
    res = bass_utils.run_bass_kernel_spmd(nc, in_maps, core_ids=list(range(NCORES)))
    return assemble(res.results)


# revision 31
# speedup vs baseline: 1.3371x; 1.0198x over previous
"""DenseAttention (causal quadratic variant, no softmax) — TRN2 Bass kernel.

Problem: out[b] = (tril(Q @ K^T) @ V) per head, where
  Q = X @ Wq (split into 16 heads of 64), K = V = X head slices.
Shapes: X [2, 2048, 1024] fp32, Wq [1024, 1024] fp32 -> out [2, 2048, 1024] fp32.

Sharding (8 cores): core c -> batch b = c//4, head group g = c%4 (4 heads,
output columns [256g, 256g+256)).  The queries projection is column-sharded
by head group; no cross-device communication.

Algorithm per core (linear-attention prefix-sum form, per head h), with
256-row outer query blocks t (T2 = 8), 128-row key blocks j:
  attnT_t = S_{<t}^T @ Q_t^T  +  sum_j V_j^T @ tril-part(K_j Q_t^T)
  S_t = S_{<t} + sum_{j in t} K_j^T V_j          ([64,64] state per head)
This reduces the strictly-causal off-diagonal work from O(N^2 hd) to
O(N hd^2).  The two heads of a "pair" p (one 128-partition group) share the
global matmul: their [64,64] states sit in a block-diagonal [128,128]
stationary tile (built by one DVE masked copy from the running Gram), so
the global term is ONE 128-contraction matmul per (t, p).  Everything is
computed transposed (attnT [hd, N]) so both matmul stages feed the tensor
engine without on-device transposes; the host un-transposes.  All matmuls
run in bf16 with fp32 PSUM accumulation; the output ships bf16.

PSUM rules honoured (hardware-verified): matmul groups writing the same
PSUM bank must share one accumulation group whose first (start=True) write
covers the region; two start=True groups split by column ranges in one
bank fault the exec unit.  Row(partition)-splits are fine.
"""

import numpy as np
import ml_dtypes

import concourse.bacc as bacc
import concourse.mybir as mybir
import concourse.tile as tile
from concourse import bass_utils
from concourse.bass import ds

B, N, D = 2, 2048, 1024
H, HD = 16, 64
NCORES = 8
P = 128           # partition dim == key block size
NQ = 256          # outer query block size
T2 = N // NQ      # 8 outer blocks
KB = N // P       # 16 key blocks
CW = 256          # per-core output column width (4 heads = 2 pairs)
CHUNK = 512       # xt chunk width

DT = mybir.dt.bfloat16
NPDT = ml_dtypes.bfloat16
F32 = mybir.dt.float32


def _emit(nc, tc, pools, xt_d, wq_d, xv_d, mk_d, out_d, deep=1):
    cpool, wpool, psq, psst, psat, pss = pools

    # ---------------- input DMAs: few, large, fully-contiguous transfers,
    # ALL on the SP (sync) HWDGE queue.  SP runs no compute, so in a looped
    # build the next emission's input DMAs issue right behind this one's
    # and prefetch during compute.  (Outputs ride the ACT queue: they are
    # produced late anyway, and must not delay the next emission's inputs.)
    wqall = cpool.tile([P, 8 * CW], DT, name="wqall", tag="wqall", bufs=deep)
    nc.sync.dma_start(out=wqall, in_=wq_d)

    xvall = cpool.tile([P, KB * CW], DT, name="xvall", tag="xvall", bufs=deep)
    xtall = cpool.tile([P, 8 * N], DT, name="xtall", tag="xtall", bufs=deep)
    mk_sb = cpool.tile([P, 640], DT, name="mk_sb", tag="mk_sb", bufs=deep)

    nc.sync.dma_start(out=xtall[:, ds(0, 4096)], in_=xt_d[:, ds(0, 4096)])
    nc.sync.dma_start(out=mk_sb, in_=mk_d)
    mk_st = mk_sb[:, ds(0, NQ + P)]      # [tril|ones|tril] (score mask)
    mk_bd = mk_sb[:, ds(NQ + P, 2 * P)]  # block-diag x2 (snapshot mask)

    for h in range(2):
        nc.sync.dma_start(
            out=xvall[:, ds(1024 * h, 1024)],
            in_=xv_d[:, ds(1024 * h, 1024)],
        )
    nc.sync.dma_start(out=xtall[:, ds(4096, 4096)], in_=xt_d[:, ds(4096, 4096)])
    for h in range(2, 4):
        nc.sync.dma_start(
            out=xvall[:, ds(1024 * h, 1024)],
            in_=xv_d[:, ds(1024 * h, 1024)],
        )
    # xt chunks 2-3 ride the ACT queue: their configs sit behind the
    # previous emission's Act compute, so they issue around this emission's
    # start -- early enough for data needed at its back half, and they
    # halve the SP queue's latency-critical stream.
    for c in range(2, 4):
        nc.scalar.dma_start(
            out=xtall[:, ds(4096 * c, 4096)],
            in_=xt_d[:, ds(4096 * c, 4096)],
        )

    def kt_ap(j, p, e):
        # K^T for (key block j, pair p, head e): [64 dims, 128 keys]
        c_, wo = divmod(P * j, CHUNK)
        return xtall[ds(HD * e, HD), ds(4096 * c_ + CHUNK * p + wo, P)]

    def xv_ap(j, col, w):
        return xvall[:, ds(CW * j + col, w)]

    qt_sb = [cpool.tile([P, N], DT, name=f"qt{m}", tag=f"qt{m}") for m in range(2)]
    otall = [cpool.tile([P, N], DT, name=f"ot{p}", tag=f"ot{p}") for p in range(2)]

    # ---------------- S states: running prefix Grams, one [128,128] matmul
    # per (pair, key block j): X_j^T @ X_j accumulated in PSUM (one tile per
    # pair -- the two pairs' groups must not share a PSUM bank).  After each
    # odd j one DVE masked-copy per pair snapshots the block-diagonal
    # [64,64] head states into a [128,128] bf16 tile (cross-head blocks
    # zeroed by the mask) for the single-matmul global term.  Grams are
    # interleaved into the main loop with a lead of ~2 key blocks so the
    # serial Gram -> snap -> Gram WAR chain always has PE work as slack.
    snaps = []   # snaps[s][p]: state over keys < 256(s+1)
    spsb = pss.tile([P, 2 * P], F32, name="spsb", tag="spsb")
    nc.vector.memset(spsb, 0.0)
    sps = [spsb[:, ds(P * p, P)] for p in range(2)]

    def emit_gram(j):
        if j > KB - 3:
            return   # blocks 14,15 are never snapshotted (diag covers them)
        for p in range(2):
            v = xv_ap(j, P * p, P)
            nc.tensor.matmul(
                sps[p], v, v, start=False, stop=(j == KB - 3),
                skip_group_check=True,
            )
        if j % 2 == 1 and j < KB - 1:
            s = j // 2
            snap = cpool.tile([P, 2 * P], DT, name=f"sn{s}", tag=f"sn{s}")
            nc.vector.tensor_mul(snap, spsb, mk_bd)
            snaps.append(snap)

    def emit_st(t):
        # scores for outer block t, per (pair, head) ONE [128,384] PSUM tile:
        # cols 0:256 = keys 2t x queries (o=0, start=True), cols 256:384 =
        # keys 2t+1 x last-half queries (o=1).  o=1 rides the SAME
        # accumulation group with start=False: o=0's bank-wide has_written
        # clear leaves o=1's cols unwritten, so its start=False write is a
        # fresh write (HW-verified semantics), avoiding a second col-split
        # start in the bank (which faults).  One DVE masked copy
        # ([tril|ones|tril]) moves the whole tile to SBUF.
        out = []
        for p in range(2):
            for e in range(2):
                stsb = wpool.tile([P, NQ + P], DT, name=f"st{t}_{p}_{e}",
                                  tag="st", bufs=6)
                stp = psst.tile([P, NQ + P], F32, name=f"sp{t}_{p}_{e}",
                                tag="big", bufs=5)
                qv0 = qt_sb[p][ds(HD * e, HD), ds(NQ * t, NQ)]
                nc.tensor.matmul(stp[:, ds(0, NQ)], kt_ap(2 * t, p, e), qv0,
                                 start=True, stop=False, skip_group_check=True)
                qv1 = qt_sb[p][ds(HD * e, HD), ds(NQ * t + P, P)]
                nc.tensor.matmul(stp[:, ds(NQ, P)], kt_ap(2 * t + 1, p, e), qv1,
                                 start=False, stop=True, skip_group_check=True)
                nc.vector.tensor_mul(stsb, stp, mk_st)
                out.append((p, e, stsb))
        return out

    def emit_pv(t, sts):
        ats = {}
        for p in range(2):
            at = psat.tile([P, NQ], F32, name=f"at{t}_{p}", tag="at", bufs=2)
            ats[p] = at
            if t > 0:
                # global term: attnT_t += S_{<t}^T @ Q_t^T (block-diag state,
                # both heads in one 128-contraction matmul; S symmetric)
                nc.tensor.matmul(
                    at, snaps[t - 1][:, ds(P * p, P)],
                    qt_sb[p][:, ds(NQ * t, NQ)],
                    start=True, stop=False, skip_group_check=True,
                )
        for p, e, stsb in sts:
            at = ats[p]
            # diagonal terms: attnT_t += V_j^T @ masked scores
            nc.tensor.matmul(
                at[ds(HD * e, HD), :],
                xv_ap(2 * t, P * p + HD * e, HD),
                stsb[:, ds(0, NQ)],
                start=(t == 0), stop=False,
                tile_position=(0, HD * e),
                skip_group_check=True,
            )
            nc.tensor.matmul(
                at[ds(HD * e, HD), ds(P, P)],
                xv_ap(2 * t + 1, P * p + HD * e, HD),
                stsb[:, ds(NQ, P)],
                start=False, stop=True,
                tile_position=(0, HD * e),
                skip_group_check=True,
            )
        for p in range(2):
            nc.scalar.copy(otall[p][:, ds(NQ * t, NQ)], ats[p])

    # ---------------- fused main loop over 512-column chunks c.
    # Q-proj runs ONE CHUNK AHEAD of the ST/PV blocks so its Act PSUM->SBUF
    # qt copy has a full stage to land before the STs read it; qp shares
    # the 5-deep "big" PSUM tag with the score tiles, so neither ever
    # waits on a fresh rotation slot.  Two-stage ST->PV pipeline as before;
    # Grams run ~4 key blocks ahead of their global-term consumers.
    def emit_qproj(c, grams=()):
        for m in range(2):
            qp = psst.tile([P, CHUNK], F32, name=f"qp{m}_{c}", tag="big",
                           bufs=5)
            for k in range(8):
                nc.tensor.matmul(
                    qp,
                    wqall[:, ds(CW * k + P * m, P)],
                    xtall[:, ds(4096 * c + CHUNK * k, CHUNK)],
                    start=(k == 0), stop=(k == 7),
                )
            nc.scalar.copy(qt_sb[m][:, ds(CHUNK * c, CHUNK)], qp)
            # prologue Grams ride between the Q-proj bursts so each
            # snapshot's Gram -> snap -> Gram chain hides under ~1.7us of
            # independent matmuls
            for j in grams[2 * m:2 * m + 2]:
                emit_gram(j)

    pending = None
    emit_qproj(0, grams=(0, 1, 2, 3))
    for c in range(4):
        if c + 1 < 4:
            emit_qproj(c + 1)
        for t in (2 * c, 2 * c + 1):
            sts = emit_st(t)
            for j in (2 * t + 4, 2 * t + 5):
                if j < KB:
                    emit_gram(j)
            if pending is not None:
                emit_pv(*pending)
            pending = (t, sts)

        if c > 0:
            for p in range(2):
                nc.gpsimd.dma_start(
                    out=out_d[ds(P * p, P), ds(CHUNK * (c - 1), CHUNK)],
                    in_=otall[p][:, ds(CHUNK * (c - 1), CHUNK)],
                )

    emit_pv(*pending)
    for p in range(2):
        nc.gpsimd.dma_start(
            out=out_d[ds(P * p, P), ds(CHUNK * 3, CHUNK)],
            in_=otall[p][:, ds(CHUNK * 3, CHUNK)],
        )


def build_nc(loop_n=1):
    nc = bacc.Bacc("TRN2", target_bir_lowering=False, debug=False)
    # all inputs ship pre-arranged in their SBUF layouts (see make_in_maps)
    xt_d = nc.dram_tensor("xt", [P, 8 * N], DT, kind="ExternalInput").ap()
    wq_d = nc.dram_tensor("wq", [P, 8 * CW], DT, kind="ExternalInput").ap()
    xv_d = nc.dram_tensor("xv", [P, KB * CW], DT, kind="ExternalInput").ap()
    mk_d = nc.dram_tensor("mk", [P, 640], DT, kind="ExternalInput").ap()
    out_d = nc.dram_tensor("outT", [CW, N], DT, kind="ExternalOutput").ap()
    unroll = 2 if loop_n > 1 else 1
    per_iter = 8 if (loop_n > 1 and loop_n % 8 == 0) else 2
    with tile.TileContext(nc) as tc:
        with (
            tc.tile_pool(name="const", bufs=unroll) as cpool,
            tc.tile_pool(name="work", bufs=1) as wpool,
            tc.tile_pool(name="psq", bufs=1, space="PSUM") as psq,
            tc.tile_pool(name="psst", bufs=1, space="PSUM") as psst,
            tc.tile_pool(name="psat", bufs=1, space="PSUM") as psat,
            tc.tile_pool(name="pss", bufs=1, space="PSUM") as pss,
        ):
            pools = (cpool, wpool, psq, psst, psat, pss)
            if loop_n > 1:
                # timing-only build: repeat the kernel on-device so the
                # per-iteration time excludes host/RPC overhead.  Two
                # emissions per For_i iteration: tag rotation gives each
                # its own buffers, so iteration i+1's input DMAs overlap
                # iteration i's compute.
                assert loop_n % per_iter == 0
                hints = (mybir.EngineType.PE, mybir.EngineType.DVE,
                         mybir.EngineType.Activation, mybir.EngineType.SP)
                with tc.For_i(0, loop_n // per_iter, 1, hint_engines=hints):
                    for _ in range(per_iter):
                        _emit(nc, tc, pools, xt_d, wq_d, xv_d, mk_d, out_d,
                              deep=2)
            else:
                _emit(nc, tc, pools, xt_d, wq_d, xv_d, mk_d, out_d)
    nc.compile()
    return nc


_CACHE = {}


def get_nc():
    if "nc" not in _CACHE:
        _CACHE["nc"] = build_nc()
    return _CACHE["nc"]


def make_in_maps(hidden_states, queries_weight):
    X = np.asarray(hidden_states, dtype=np.float32)
    W = np.asarray(queries_weight, dtype=np.float32)
    r = np.arange(P)[:, None]
    c = np.arange(NQ)[None, :]
    m0 = (c >= r).astype(np.float32)                        # [tril | ones]
    bd = np.zeros((P, P), dtype=np.float32)                 # block-diag ones
    bd[:HD, :HD] = 1.0
    bd[HD:, HD:] = 1.0
    tril = m0[:, :P]
    mk = np.concatenate([m0, tril, bd, bd], axis=1).astype(NPDT)  # [128, 640]
    in_maps = []
    for core in range(NCORES):
        b, g = divmod(core, 4)
        cols = slice(CW * g, CW * g + CW)
        # Permute the contraction rows so every core sees its own heads'
        # K^T rows at xt rows [0, 256) (keeps the program core-agnostic).
        perm = np.r_[
            np.arange(CW * g, CW * g + CW),
            np.arange(0, CW * g),
            np.arange(CW * g + CW, D),
        ]
        # pre-arrange into SBUF layouts so every DMA is fully contiguous:
        #   xt: [p, (chunk c, k-tile, w)], wq: [p, (k, w)], xv: [p, (j, w)]
        xt = (X[b].T[perm].reshape(8, P, 4, CHUNK).transpose(1, 2, 0, 3)
              .reshape(P, 8 * N))
        wq = W[perm][:, cols].reshape(8, P, CW).transpose(1, 0, 2).reshape(P, 8 * CW)
        xv = X[b][:, cols].reshape(KB, P, CW).transpose(1, 0, 2).reshape(P, KB * CW)
        in_maps.append({
            "xt": np.ascontiguousarray(xt).astype(NPDT),
            "wq": np.ascontiguousarray(wq).astype(NPDT),
            "xv": np.ascontiguousarray(xv).astype(NPDT),
            "mk": mk,
        })
    return in_maps


def assemble(results):
    out = np.empty((B, N, D), dtype=np.float32)
    for core in range(NCORES):
        b, g = divmod(core, 4)
        out[b, :, CW * g:CW * g + CW] = results[core]["outT"].astype(np.float32).T
    return out


def kernel(hidden_states, queries_weight):
    nc = get_nc()
    in_maps = make_in_maps(hidden_states, queries_weight)
    res = bass_utils.run_bass_kernel_spmd(nc, in_maps, core_ids=list(range(NCORES)))
    return assemble(res.results)


# revision 32
# speedup vs baseline: 1.3944x; 1.0428x over previous
"""DenseAttention (causal quadratic variant, no softmax) — TRN2 Bass kernel.

Problem: out[b] = (tril(Q @ K^T) @ V) per head, where
  Q = X @ Wq (split into 16 heads of 64), K = V = X head slices.
Shapes: X [2, 2048, 1024] fp32, Wq [1024, 1024] fp32 -> out [2, 2048, 1024] fp32.

Sharding (8 cores): core c -> batch b = c//4, head group g = c%4 (4 heads,
output columns [256g, 256g+256)).  The queries projection is column-sharded
by head group; no cross-device communication.

Algorithm per core (linear-attention prefix-sum form, per head h), with
256-row outer query blocks t (T2 = 8), 128-row key blocks j:
  attnT_t = S_{<t}^T @ Q_t^T  +  sum_j V_j^T @ tril-part(K_j Q_t^T)
  S_t = S_{<t} + sum_{j in t} K_j^T V_j          ([64,64] state per head)
This reduces the strictly-causal off-diagonal work from O(N^2 hd) to
O(N hd^2).  The two heads of a "pair" p (one 128-partition group) share the
global matmul: their [64,64] states sit in a block-diagonal [128,128]
stationary tile (built by one DVE masked copy from the running Gram), so
the global term is ONE 128-contraction matmul per (t, p).  Everything is
computed transposed (attnT [hd, N]) so both matmul stages feed the tensor
engine without on-device transposes; the host un-transposes.  All matmuls
run in bf16 with fp32 PSUM accumulation; the output ships bf16.

PSUM rules honoured (hardware-verified): matmul groups writing the same
PSUM bank must share one accumulation group whose first (start=True) write
covers the region; two start=True groups split by column ranges in one
bank fault the exec unit.  Row(partition)-splits are fine.
"""

import numpy as np
import ml_dtypes

import concourse.bacc as bacc
import concourse.mybir as mybir
import concourse.tile as tile
from concourse import bass_utils
from concourse.bass import ds

B, N, D = 2, 2048, 1024
H, HD = 16, 64
NCORES = 8
P = 128           # partition dim == key block size
NQ = 256          # outer query block size
T2 = N // NQ      # 8 outer blocks
KB = N // P       # 16 key blocks
CW = 256          # per-core output column width (4 heads = 2 pairs)
CHUNK = 512       # xt chunk width

DT = mybir.dt.bfloat16
NPDT = ml_dtypes.bfloat16
F32 = mybir.dt.float32


def _emit(nc, tc, pools, xt_d, wq_d, xv_d, mk_d, out_d, deep=1):
    cpool, wpool, psq, psst, psat, pss = pools

    # ---------------- input DMAs: few, large, fully-contiguous transfers,
    # ALL on the SP (sync) HWDGE queue.  SP runs no compute, so in a looped
    # build the next emission's input DMAs issue right behind this one's
    # and prefetch during compute.  (Outputs ride the ACT queue: they are
    # produced late anyway, and must not delay the next emission's inputs.)
    wqall = cpool.tile([P, 8 * CW], DT, name="wqall", tag="wqall", bufs=deep)
    nc.sync.dma_start(out=wqall, in_=wq_d)

    xvall = cpool.tile([P, KB * CW], DT, name="xvall", tag="xvall", bufs=deep)
    xtall = cpool.tile([P, 8 * N], DT, name="xtall", tag="xtall", bufs=deep)
    mk_sb = cpool.tile([P, 640], DT, name="mk_sb", tag="mk_sb", bufs=deep)

    nc.sync.dma_start(out=xtall[:, ds(0, 4096)], in_=xt_d[:, ds(0, 4096)])
    nc.sync.dma_start(out=mk_sb, in_=mk_d)
    mk_st = mk_sb[:, ds(0, NQ + P)]      # [tril|ones|tril] (score mask)
    mk_bd = mk_sb[:, ds(NQ + P, 2 * P)]  # block-diag x2 (snapshot mask)

    for h in range(2):
        nc.sync.dma_start(
            out=xvall[:, ds(1024 * h, 1024)],
            in_=xv_d[:, ds(1024 * h, 1024)],
        )
    nc.sync.dma_start(out=xtall[:, ds(4096, 4096)], in_=xt_d[:, ds(4096, 4096)])
    for h in range(2, 4):
        nc.sync.dma_start(
            out=xvall[:, ds(1024 * h, 1024)],
            in_=xv_d[:, ds(1024 * h, 1024)],
        )
    # xt chunks 2-3 ride the ACT queue: their configs sit behind the
    # previous emission's Act compute, so they issue around this emission's
    # start -- early enough for data needed at its back half, and they
    # halve the SP queue's latency-critical stream.
    for c in range(2, 4):
        nc.scalar.dma_start(
            out=xtall[:, ds(4096 * c, 4096)],
            in_=xt_d[:, ds(4096 * c, 4096)],
        )

    def kt_ap(j, p, e):
        # K^T for (key block j, pair p, head e): [64 dims, 128 keys]
        c_, wo = divmod(P * j, CHUNK)
        return xtall[ds(HD * e, HD), ds(4096 * c_ + CHUNK * p + wo, P)]

    def xv_ap(j, col, w):
        return xvall[:, ds(CW * j + col, w)]

    qt_sb = [cpool.tile([P, N], DT, name=f"qt{m}", tag=f"qt{m}") for m in range(2)]
    otall = [cpool.tile([P, N], DT, name=f"ot{p}", tag=f"ot{p}") for p in range(2)]

    # ---------------- S states: running prefix Grams, one [128,128] matmul
    # per (pair, key block j): X_j^T @ X_j accumulated in PSUM (one tile per
    # pair -- the two pairs' groups must not share a PSUM bank).  After each
    # odd j one DVE masked-copy per pair snapshots the block-diagonal
    # [64,64] head states into a [128,128] bf16 tile (cross-head blocks
    # zeroed by the mask) for the single-matmul global term.  Grams are
    # interleaved into the main loop with a lead of ~2 key blocks so the
    # serial Gram -> snap -> Gram WAR chain always has PE work as slack.
    snaps = []   # snaps[s][p]: state over keys < 256(s+1)
    spsb = pss.tile([P, 2 * P], F32, name="spsb", tag="spsb")
    nc.vector.memset(spsb, 0.0)
    sps = [spsb[:, ds(P * p, P)] for p in range(2)]

    def emit_gram(j):
        if j > KB - 3:
            return   # blocks 14,15 are never snapshotted (diag covers them)
        for p in range(2):
            v = xv_ap(j, P * p, P)
            nc.tensor.matmul(
                sps[p], v, v, start=False, stop=(j == KB - 3),
                skip_group_check=True,
            )
        if j % 2 == 1 and j < KB - 1:
            s = j // 2
            snap = cpool.tile([P, 2 * P], DT, name=f"sn{s}", tag=f"sn{s}")
            nc.vector.tensor_mul(snap, spsb, mk_bd)
            snaps.append(snap)

    def emit_st(t):
        # scores for outer block t, per (pair, head) ONE [128,384] PSUM tile:
        # cols 0:256 = keys 2t x queries (o=0, start=True), cols 256:384 =
        # keys 2t+1 x last-half queries (o=1).  o=1 rides the SAME
        # accumulation group with start=False: o=0's bank-wide has_written
        # clear leaves o=1's cols unwritten, so its start=False write is a
        # fresh write (HW-verified semantics), avoiding a second col-split
        # start in the bank (which faults).  One DVE masked copy
        # ([tril|ones|tril]) moves the whole tile to SBUF.
        out = []
        for p in range(2):
            for e in range(2):
                stsb = wpool.tile([P, NQ + P], DT, name=f"st{t}_{p}_{e}",
                                  tag="st", bufs=8)
                stp = psst.tile([P, NQ + P], F32, name=f"sp{t}_{p}_{e}",
                                tag="big", bufs=5)
                qv0 = qt_sb[p][ds(HD * e, HD), ds(NQ * t, NQ)]
                nc.tensor.matmul(stp[:, ds(0, NQ)], kt_ap(2 * t, p, e), qv0,
                                 start=True, stop=False, skip_group_check=True)
                qv1 = qt_sb[p][ds(HD * e, HD), ds(NQ * t + P, P)]
                nc.tensor.matmul(stp[:, ds(NQ, P)], kt_ap(2 * t + 1, p, e), qv1,
                                 start=False, stop=True, skip_group_check=True)
                nc.vector.tensor_mul(stsb, stp, mk_st)
                out.append((p, e, stsb))
        return out

    def emit_pv(t, sts):
        ats = {}
        for p in range(2):
            at = psat.tile([P, NQ], F32, name=f"at{t}_{p}", tag="at", bufs=2)
            ats[p] = at
            if t > 0:
                # global term: attnT_t += S_{<t}^T @ Q_t^T (block-diag state,
                # both heads in one 128-contraction matmul; S symmetric)
                nc.tensor.matmul(
                    at, snaps[t - 1][:, ds(P * p, P)],
                    qt_sb[p][:, ds(NQ * t, NQ)],
                    start=True, stop=False, skip_group_check=True,
                )
        for p, e, stsb in sts:
            at = ats[p]
            # diagonal terms: attnT_t += V_j^T @ masked scores
            nc.tensor.matmul(
                at[ds(HD * e, HD), :],
                xv_ap(2 * t, P * p + HD * e, HD),
                stsb[:, ds(0, NQ)],
                start=(t == 0), stop=False,
                tile_position=(0, HD * e),
                skip_group_check=True,
            )
            nc.tensor.matmul(
                at[ds(HD * e, HD), ds(P, P)],
                xv_ap(2 * t + 1, P * p + HD * e, HD),
                stsb[:, ds(NQ, P)],
                start=False, stop=True,
                tile_position=(0, HD * e),
                skip_group_check=True,
            )
        for p in range(2):
            nc.scalar.copy(otall[p][:, ds(NQ * t, NQ)], ats[p])

    # ---------------- fused main loop over 512-column chunks c.
    # Q-proj runs ONE CHUNK AHEAD of the ST/PV blocks so its Act PSUM->SBUF
    # qt copy has a full stage to land before the STs read it; qp shares
    # the 5-deep "big" PSUM tag with the score tiles, so neither ever
    # waits on a fresh rotation slot.  Two-stage ST->PV pipeline as before;
    # Grams run ~4 key blocks ahead of their global-term consumers.
    def emit_qproj(c, grams=()):
        for m in range(2):
            qp = psst.tile([P, CHUNK], F32, name=f"qp{m}_{c}", tag="big",
                           bufs=5)
            for k in range(8):
                nc.tensor.matmul(
                    qp,
                    wqall[:, ds(CW * k + P * m, P)],
                    xtall[:, ds(4096 * c + CHUNK * k, CHUNK)],
                    start=(k == 0), stop=(k == 7),
                )
            nc.scalar.copy(qt_sb[m][:, ds(CHUNK * c, CHUNK)], qp)
            # prologue Grams ride between the Q-proj bursts so each
            # snapshot's Gram -> snap -> Gram chain hides under ~1.7us of
            # independent matmuls
            for j in grams[2 * m:2 * m + 2]:
                emit_gram(j)

    pending = None
    emit_qproj(0, grams=(0, 1, 2, 3))
    for c in range(4):
        if c + 1 < 4:
            emit_qproj(c + 1)
        for t in (2 * c, 2 * c + 1):
            sts = emit_st(t)
            for j in (2 * t + 4, 2 * t + 5):
                if j < KB:
                    emit_gram(j)
            if pending is not None:
                emit_pv(*pending)
            pending = (t, sts)

        if c > 0:
            for p in range(2):
                nc.gpsimd.dma_start(
                    out=out_d[ds(P * p, P), ds(CHUNK * (c - 1), CHUNK)],
                    in_=otall[p][:, ds(CHUNK * (c - 1), CHUNK)],
                )

    emit_pv(*pending)
    for p in range(2):
        nc.gpsimd.dma_start(
            out=out_d[ds(P * p, P), ds(CHUNK * 3, CHUNK)],
            in_=otall[p][:, ds(CHUNK * 3, CHUNK)],
        )


def build_nc(loop_n=1):
    nc = bacc.Bacc("TRN2", target_bir_lowering=False, debug=False)
    # all inputs ship pre-arranged in their SBUF layouts (see make_in_maps)
    xt_d = nc.dram_tensor("xt", [P, 8 * N], DT, kind="ExternalInput").ap()
    wq_d = nc.dram_tensor("wq", [P, 8 * CW], DT, kind="ExternalInput").ap()
    xv_d = nc.dram_tensor("xv", [P, KB * CW], DT, kind="ExternalInput").ap()
    mk_d = nc.dram_tensor("mk", [P, 640], DT, kind="ExternalInput").ap()
    out_d = nc.dram_tensor("outT", [CW, N], DT, kind="ExternalOutput").ap()
    unroll = 2 if loop_n > 1 else 1
    per_iter = 16 if (loop_n > 1 and loop_n % 16 == 0) else 2
    with tile.TileContext(nc) as tc:
        with (
            tc.tile_pool(name="const", bufs=unroll) as cpool,
            tc.tile_pool(name="work", bufs=1) as wpool,
            tc.tile_pool(name="psq", bufs=1, space="PSUM") as psq,
            tc.tile_pool(name="psst", bufs=1, space="PSUM") as psst,
            tc.tile_pool(name="psat", bufs=1, space="PSUM") as psat,
            tc.tile_pool(name="pss", bufs=1, space="PSUM") as pss,
        ):
            pools = (cpool, wpool, psq, psst, psat, pss)
            if loop_n > 1:
                # timing-only build: repeat the kernel on-device so the
                # per-iteration time excludes host/RPC overhead.  Two
                # emissions per For_i iteration: tag rotation gives each
                # its own buffers, so iteration i+1's input DMAs overlap
                # iteration i's compute.
                assert loop_n % per_iter == 0
                hints = (mybir.EngineType.PE, mybir.EngineType.DVE,
                         mybir.EngineType.Activation, mybir.EngineType.SP)
                with tc.For_i(0, loop_n // per_iter, 1, hint_engines=hints):
                    for _ in range(per_iter):
                        _emit(nc, tc, pools, xt_d, wq_d, xv_d, mk_d, out_d,
                              deep=2)
            else:
                _emit(nc, tc, pools, xt_d, wq_d, xv_d, mk_d, out_d)
    nc.compile()
    return nc


_CACHE = {}


def get_nc():
    if "nc" not in _CACHE:
        _CACHE["nc"] = build_nc()
    return _CACHE["nc"]


def make_in_maps(hidden_states, queries_weight):
    X = np.asarray(hidden_states, dtype=np.float32)
    W = np.asarray(queries_weight, dtype=np.float32)
    r = np.arange(P)[:, None]
    c = np.arange(NQ)[None, :]
    m0 = (c >= r).astype(np.float32)                        # [tril | ones]
    bd = np.zeros((P, P), dtype=np.float32)                 # block-diag ones
    bd[:HD, :HD] = 1.0
    bd[HD:, HD:] = 1.0
    tril = m0[:, :P]
    mk = np.concatenate([m0, tril, bd, bd], axis=1).astype(NPDT)  # [128, 640]
    in_maps = []
    for core in range(NCORES):
        b, g = divmod(core, 4)
        cols = slice(CW * g, CW * g + CW)
        # Permute the contraction rows so every core sees its own heads'
        # K^T rows at xt rows [0, 256) (keeps the program core-agnostic).
        perm = np.r_[
            np.arange(CW * g, CW * g + CW),
            np.arange(0, CW * g),
            np.arange(CW * g + CW, D),
        ]
        # pre-arrange into SBUF layouts so every DMA is fully contiguous:
        #   xt: [p, (chunk c, k-tile, w)], wq: [p, (k, w)], xv: [p, (j, w)]
        xt = (X[b].T[perm].reshape(8, P, 4, CHUNK).transpose(1, 2, 0, 3)
              .reshape(P, 8 * N))
        wq = W[perm][:, cols].reshape(8, P, CW).transpose(1, 0, 2).reshape(P, 8 * CW)
        xv = X[b][:, cols].reshape(KB, P, CW).transpose(1, 0, 2).reshape(P, KB * CW)
        in_maps.append({
            "xt": np.ascontiguousarray(xt).astype(NPDT),
            "wq": np.ascontiguousarray(wq).astype(NPDT),
            "xv": np.ascontiguousarray(xv).astype(NPDT),
            "mk": mk,
        })
    return in_maps


def assemble(results):
    out = np.empty((B, N, D), dtype=np.float32)
    for core in range(NCORES):
        b, g = divmod(core, 4)
        out[b, :, CW * g:CW * g + CW] = results[core]["outT"].astype(np.float32).T
    return out


def kernel(hidden_states, queries_weight):
    nc = get_nc()
    in_maps = make_in_maps(hidden_states, queries_weight)
    res = bass_utils.run_bass_kernel_spmd(nc, in_maps, core_ids=list(range(NCORES)))
    return assemble(res.results)
